# revision 1
# baseline (speedup 1.0000x reference)
"""LIF spiking-neuron recurrence on Trainium2 (8 NeuronCores) — hybrid PE+DVE.

Reference semantics (TAU=1, THRESH=1, f32):
    mem = 0
    for t in range(T):
        mem = mem + x[t]
        spike[t] = (mem >= 1.0) ? 1.0 : 0.0
        mem = mem * (1 - spike[t])        # hard reset

Sharding: data-parallel over batch (B=128 -> 16 rows/core); per-core
[T, 16, 16384] viewed as [T, 128, 2048], host-transposed to [128, T, 2048].

The baseline (two DVE ops/step over the full 2048-wide slab) is
DVE-bound at ~4.7us/step. This kernel splits the slab by column so the
tensor engine absorbs part of the serial add chain:

  DVE cols [0:1536):  tmp = mem_d + x_t (TT);  mem_d = (tmp<1)*tmp (STT)
  PE  cols [1536:2048): pre = I@x_t (start=True) + I@mem_p (accumulate)
      into f32 PSUM - probed bit-exact (identity f32 matmul and PSUM
      accumulate == RN(a+b); only -0.0 -> +0.0, which never changes
      values or comparisons); mem_p = mask_u8 * pre (TT, u8*f32 probed
      exact; spiking implies pre>=1>0 so mask*pre reset yields +0.0).
  ACT: mask = Sign(1 - pre) -> uint8 for ALL columns == [pre < 1]
      EXACTLY including threshold ties (Sign(0)=0; probed: negative
      Sign saturates to u8 0). Host emits spikes = 1 - mask. The input
      data contains one exact pre==1.0 tie; this path gets it right.

Scheduling notes (hard-won, all measured on HW):
  * f32 matmuls lower to 2 physical MM+LDWEIGHTS pairs; a logical
    512-col f32 MM costs ~1.66us warm, ~2.4us cold. FP=512 (one
    512-col chunk) is the most the PE can carry: its per-step chain
    (mem-MM -> Sign -> mask-mult reset -> next mem-MM) must hide
    under the DVE's ~4.2us of per-step work.
  * PE HAM drops to 1.2GHz after ~3.4us idle windows; cold matmuls
    double the chain latency. A 24-MM bf16 priming preamble plus one
    tiny bf16 dummy matmul per step (emitted BEFORE the chain-critical
    mem-MM so the in-order PE queue never delays it) keeps it at 2.4GHz.
  * x-matmuls for step t+1 are emitted one step ahead (dep-free) to
    fill PE wait gaps; PSUM pre is triple-buffered to decouple them.
  * ACT emits the PE-column Sign before the DVE-column Sign: only the
    former is on the cross-engine chain.
  * Injecting x into PSUM via ACT Copy (to drop the x-matmul) measures
    far SLOWER in this loop despite lower op counts - ScalarE PSUM
    writes + PE accumulates + DVE PSUM reads do not compose well.

Measured: ~280us HW exec (neuron-profile, core 0), bit-exact vs the
jax f32 reference (vs 320us for the all-DVE baseline). DVE busy ~89%
(~256us: TT 1757ns + STT 1749ns + 512-col mask-mult 678ns per step);
per-core DMA is 84MB (64MiB f32 in, 16MiB u8 out) vs the ~358GB/s
HBM-per-core limit, so ~235us is the DMA floor of any variant keeping
u8 outputs.
"""

import numpy as np

try:
    import concourse  # noqa: F401
except ImportError:  # pragma: no cover
    import sys

    for _p in ("/opt/trn_rl_repo", "/root/.axon_site/_ro/trn_rl_repo"):
        if _p not in sys.path:
            sys.path.insert(0, _p)

from concourse import bacc, mybir
from concourse.bass_utils import run_bass_kernel_spmd
from concourse.mybir import ActivationFunctionType as AF
from concourse.mybir import AluOpType
from concourse.tile import TileContext

T, B, D = 64, 128, 16384
NCORES = 8
BL = B // NCORES  # 16 batch rows per core
P = 128
F = (BL * D) // P  # 2048
FP = 512  # PE/PSUM-path columns
FD = F - FP  # pure-DVE-path columns
CHUNK = 4  # timesteps per DMA transfer
NMM = FP // 512  # 512-col matmuls per operand pass
NPS = 3  # PSUM pre buffers (2 banks each)


def build_nc(t_steps=T, chunk=CHUNK, x_bufs=3, s_bufs=3, t_bufs=3):
    assert t_steps % chunk == 0
    f32 = mybir.dt.float32
    u8 = mybir.dt.uint8
    nc = bacc.Bacc(
        "TRN2", target_bir_lowering=False, debug=False, num_devices=NCORES
    )
    x_ext = nc.dram_tensor("x", [P, t_steps, F], f32, kind="ExternalInput")
    id_ext = nc.dram_tensor("ident", [P, P], f32, kind="ExternalInput")
    out_ext = nc.dram_tensor(
        "out", [P, t_steps, F], u8, kind="ExternalOutput"
    )
    n_groups = t_steps // chunk
    with TileContext(nc) as tc:
        with (
            tc.tile_pool(name="mp", bufs=1) as mp,
            tc.tile_pool(name="tp", bufs=t_bufs) as tp,
            tc.tile_pool(name="xp", bufs=x_bufs) as xp,
            tc.tile_pool(name="sp", bufs=s_bufs) as sp,
            tc.psum_pool(name="pp", bufs=1) as pp,
        ):
            mem_d = mp.tile([P, FD], f32, name="mem_d")
            mem_p = mp.tile([P, FP], f32, name="mem_p")
            ident = mp.tile([P, P], f32, name="ident")
            b1 = mp.tile([P, 1], f32, name="b1")
            nc.vector.memset(b1[:], 1.0)
            nc.sync.dma_start(ident[:], id_ext[:, :])
            ps = [pp.tile([P, FP], f32, name=f"ps{i}") for i in range(NPS)]
            scr = pp.tile([P, 512], f32, name="scr")  # dummy-MM target

            def warm_mm(src):
                # tiny bf16 matmul into scratch: keeps the PE HAM busy so the
                # chain-critical f32 mem-matmuls run at 2.4GHz; result unused
                nc.tensor.matmul(
                    scr[:, 0:256],
                    ident[:, 0:64].bitcast(mybir.dt.bfloat16),
                    src[:, 0:128].bitcast(mybir.dt.bfloat16),
                    start=True,
                    stop=True,
                )

            x_tiles = {}

            def x_slab(t):
                g, j = divmod(t, chunk)
                return x_tiles[g][:, j * F : (j + 1) * F]

            def emit_x_mms(t):
                """pre[t] = I @ x_t[PE cols] (opens the accumulation group)."""
                pre = ps[t % NPS]
                xs_p = x_slab(t)[:, FD:F]
                for c in range(NMM):
                    sl = slice(c * 512, (c + 1) * 512)
                    nc.tensor.matmul(
                        pre[:, sl], ident[:], xs_p[:, sl],
                        start=True, stop=(t == 0),
                    )

            def ensure_x_loaded(g):
                if g in x_tiles or g >= n_groups:
                    return
                xt = xp.tile([P, chunk * F], f32, name="xt")
                x_tiles[g] = xt
                xv = x_ext[:, g * chunk : (g + 1) * chunk, :]
                for j in range(chunk):
                    nc.sync.dma_start(
                        xt[:, j * F : (j + 1) * F], xv[:, j, :]
                    )

            for _ in range(24):  # prime the PE HAM to full clock
                warm_mm(ident)

            for g in range(n_groups):
                ensure_x_loaded(g)
                ensure_x_loaded(g + 1)
                spk = sp.tile([P, chunk * F], u8, name="spk")
                for j in range(chunk):
                    t = g * chunk + j
                    pre = ps[t % NPS]
                    xs_d = x_slab(t)[:, 0:FD]
                    mk_d = spk[:, j * F : j * F + FD]
                    mk_p = spk[:, j * F + FD : (j + 1) * F]

                    if t == 0:
                        emit_x_mms(0)
                    warm_mm(x_slab(t))
                    # close this step's PSUM accumulation: pre += I @ mem_p
                    if t > 0:
                        for c in range(NMM):
                            sl = slice(c * 512, (c + 1) * 512)
                            nc.tensor.matmul(
                                pre[:, sl], ident[:],
                                mem_p[:, sl],
                                start=False, stop=True,
                            )
                    # keep PE busy: next step's dep-free x matmuls
                    if t + 1 < t_steps:
                        emit_x_mms(t + 1)

                    # ---- ACT: PE-col masks first (critical chain) ----
                    for c in range(NMM):
                        sl = slice(c * 512, (c + 1) * 512)
                        nc.scalar.activation(
                            mk_p[:, sl], pre[:, sl], AF.Sign,
                            bias=b1[:], scale=-1.0,
                        )

                    # ---- DVE columns: TT + STT ----
                    if t == 0:
                        tmp_s = xs_d
                    else:
                        tmp = tp.tile([P, FD], f32, name="tmp")
                        nc.vector.tensor_tensor(
                            tmp[:], mem_d[:], xs_d, AluOpType.add
                        )
                        tmp_s = tmp[:]
                    nc.scalar.activation(
                        mk_d, tmp_s, AF.Sign, bias=b1[:], scale=-1.0
                    )
                    if t < t_steps - 1:
                        nc.vector.scalar_tensor_tensor(
                            mem_d[:], tmp_s, 1.0, tmp_s,
                            AluOpType.is_lt, AluOpType.mult,
                        )
                        # PE-col reset: mem_p = mask * pre (u8 * f32, exact)
                        for c in range(NMM):
                            sl = slice(c * 512, (c + 1) * 512)
                            nc.vector.tensor_tensor(
                                mem_p[:, sl], mk_p[:, sl], pre[:, sl],
                                AluOpType.mult,
                            )
                    if g == n_groups - 1:
                        nc.scalar.dma_start(
                            out_ext[:, g * chunk + j, :],
                            spk[:, j * F : (j + 1) * F],
                        )
                if g < n_groups - 1:
                    nc.scalar.dma_start(
                        out_ext[:, g * chunk : (g + 1) * chunk, :].rearrange(
                            "p t f -> p (t f)"
                        ),
                        spk[:],
                    )
    nc.compile()
    return nc


_cached_nc = None


def _get_nc():
    global _cached_nc
    if _cached_nc is None:
        _cached_nc = build_nc()
    return _cached_nc


_IDENT = np.eye(P, dtype=np.float32)


def _shard(x):
    in_maps = []
    for c in range(NCORES):
        xc = x[:, c * BL : (c + 1) * BL, :].reshape(T, P, F).transpose(1, 0, 2)
        in_maps.append({"x": np.ascontiguousarray(xc), "ident": _IDENT})
    return in_maps


def _gather(results):
    outs = [
        (1 - np.asarray(results[c]["out"]))
        .astype(np.float32)
        .transpose(1, 0, 2)
        .reshape(T, BL, D)
        for c in range(NCORES)
    ]
    return np.concatenate(outs, axis=1)


def run(x, trace=False, **kw):
    x = np.ascontiguousarray(np.asarray(x, dtype=np.float32))
    assert x.shape == (T, B, D), x.shape
    nc = _get_nc()
    res = run_bass_kernel_spmd(
        nc, _shard(x), core_ids=list(range(NCORES)), trace=trace, **kw
    )
    return _gather(res.results), res


def kernel(x: np.ndarray) -> np.ndarray:
    out, _ = run(x)
    return out



# revision 2
# speedup vs baseline: 1.1449x; 1.1449x over previous
"""LIF spiking-neuron recurrence on Trainium2 (8 NeuronCores) — fused DVE op.

Reference semantics (TAU=1, THRESH=1, f32):
    mem = 0
    for t in range(T):
        mem = mem + x[t]
        spike[t] = (mem >= 1.0) ? 1.0 : 0.0
        mem = mem * (1 - spike[t])        # hard reset

Sharding: data-parallel over batch (B=128 -> 16 rows/core); per-core
[T, 16, 16384] viewed as [T, 128, 2048], host-transposed to [128, T, 2048].

Core trick: a runtime-registered custom DVE op fuses the previous step's
reset into this step's add, so the recurrence is ONE 1x DVE pass/step:

    tmp_t = select(tmp_{t-1} < 1, tmp_{t-1}, 0) + x_t      (LIF_STEP_ANT)

tmp_t is the pre-reset membrane; mem_t is never materialized. Bit-exact:
select passes tmp through untouched (incl -0.0) below threshold and
yields +0.0 at/above it, matching tmp*(1-spike) in f32. ACT computes
mask_t = [tmp_t < 1] as u8 via Sign(1 - tmp_t) (negative Sign saturates
to u8 0; Sign(0)=0 handles the exact tie tmp==1). Host: spikes = 1-mask.

DVE/step: (2048+151)/0.96 = 2.29us; ACT: (2048+352)/1.2 = 2.0us in
parallel. Compute ~150us; per-core DMA 84MB at ~358GB/s -> ~235us floor,
so this kernel is DMA-bound (was DVE-bound at ~280us with the old
PE+DVE hybrid).
"""

import numpy as np

try:
    import concourse  # noqa: F401
except ImportError:  # pragma: no cover
    import sys

    for _p in ("/opt/trn_rl_repo", "/root/.axon_site/_ro/trn_rl_repo"):
        if _p not in sys.path:
            sys.path.insert(0, _p)

from concourse import bacc, mybir
from concourse.bass_utils import run_bass_kernel_spmd
from concourse.mybir import ActivationFunctionType as AF
from concourse.tile import TileContext

T, B, D = 64, 128, 16384
NCORES = 8
BL = B // NCORES  # 16 batch rows per core
P = 128
F = (BL * D) // P  # 2048
CHUNK = 4  # timesteps per output DMA group


def register_lif_op():
    """Runtime-register the fused LIF-step DVE op:
    out = select(in0 < s0, in0, 0) + in1  (s0 = threshold)."""
    from concourse import dve_ops
    from concourse.dve_ops import (
        OPS,
        DveOp,
        _CUSTOM_DVE_ROW_BASE,
        _SUB_OPCODE_FOR_NAME,
    )
    from concourse.dve_spec import C0, Spec, Src0, Src1, Zero, select

    if "LIF_STEP_ANT" in _SUB_OPCODE_FOR_NAME:
        return next(op for op in OPS if op.name == "LIF_STEP_ANT")

    spec = Spec(
        body=select(Src0 < C0, Src0, Zero) + Src1,
        reference=lambda in0, in1, s0, s1, imm2: (
            np.where(in0 < s0, in0, np.float32(0.0)) + in1
        ).astype(np.float32),
    )
    op = DveOp(
        "LIF_STEP_ANT",
        spec,
        subdim=False,
        uops_sha={"v3": "38f6b55dbeb193f6", "v4": "cb4fb9e0c41a0972"},
    )
    OPS.append(op)
    _SUB_OPCODE_FOR_NAME[op.name] = _CUSTOM_DVE_ROW_BASE + len(OPS) - 1
    dve_ops.CUSTOM_DVE_SPECS[op.name] = op.spec
    return op


def build_nc(t_steps=T, chunk=CHUNK, x_bufs=3, s_bufs=3):
    assert t_steps % chunk == 0
    f32 = mybir.dt.float32
    u8 = mybir.dt.uint8
    lif = register_lif_op()
    nc = bacc.Bacc(
        "TRN2", target_bir_lowering=False, debug=False, num_devices=NCORES
    )
    x_ext = nc.dram_tensor("x", [P, t_steps, F], f32, kind="ExternalInput")
    out_ext = nc.dram_tensor(
        "out", [P, t_steps, F], u8, kind="ExternalOutput"
    )
    n_groups = t_steps // chunk
    with TileContext(nc) as tc:
        with (
            tc.tile_pool(name="mp", bufs=1) as mp,
            tc.tile_pool(name="tp", bufs=3) as tp,
            tc.tile_pool(name="xp", bufs=x_bufs) as xp,
            tc.tile_pool(name="sp", bufs=s_bufs) as sp,
        ):
            b1 = mp.tile([P, 1], f32, name="b1")
            nc.vector.memset(b1[:], 1.0)
            zero = mp.tile([P, F], f32, name="zero")
            nc.vector.memset(zero[:], 0.0)

            x_tiles = {}

            def x_slab(t):
                g, j = divmod(t, chunk)
                return x_tiles[g][:, j * F : (j + 1) * F]

            def ensure_x_loaded(g):
                if g in x_tiles or g >= n_groups:
                    return
                xt = xp.tile([P, chunk * F], f32, name="xt")
                x_tiles[g] = xt
                xv = x_ext[:, g * chunk : (g + 1) * chunk, :]
                for j in range(chunk):
                    nc.sync.dma_start(
                        xt[:, j * F : (j + 1) * F], xv[:, j, :]
                    )

            prev = zero
            for g in range(n_groups):
                ensure_x_loaded(g)
                ensure_x_loaded(g + 1)
                spk = sp.tile([P, chunk * F], u8, name="spk")
                for j in range(chunk):
                    t = g * chunk + j
                    tmp = tp.tile([P, F], f32, name="tmp")
                    # tmp_t = select(tmp_{t-1} < 1, tmp_{t-1}, 0) + x_t
                    nc.vector._custom_dve(
                        lif, out=tmp[:], in0=prev[:], in1=x_slab(t), s0=1.0
                    )
                    # mask_t = [tmp_t < 1] as u8 (Sign(1-tmp); <0 saturates to 0)
                    nc.scalar.activation(
                        spk[:, j * F : (j + 1) * F], tmp[:], AF.Sign,
                        bias=b1[:], scale=-1.0,
                    )
                    prev = tmp
                nc.scalar.dma_start(
                    out_ext[:, g * chunk : (g + 1) * chunk, :].rearrange(
                        "p t f -> p (t f)"
                    ),
                    spk[:],
                )
    nc.compile()
    return nc


_cached_nc = None


def _get_nc():
    global _cached_nc
    if _cached_nc is None:
        _cached_nc = build_nc()
    return _cached_nc


def _shard(x):
    in_maps = []
    for c in range(NCORES):
        xc = x[:, c * BL : (c + 1) * BL, :].reshape(T, P, F).transpose(1, 0, 2)
        in_maps.append({"x": np.ascontiguousarray(xc)})
    return in_maps


def _gather(results):
    outs = [
        (1 - np.asarray(results[c]["out"]))
        .astype(np.float32)
        .transpose(1, 0, 2)
        .reshape(T, BL, D)
        for c in range(NCORES)
    ]
    return np.concatenate(outs, axis=1)


def run(x, trace=False, **kw):
    x = np.ascontiguousarray(np.asarray(x, dtype=np.float32))
    assert x.shape == (T, B, D), x.shape
    nc = _get_nc()
    res = run_bass_kernel_spmd(
        nc, _shard(x), core_ids=list(range(NCORES)), trace=trace, **kw
    )
    return _gather(res.results), res


def kernel(x: np.ndarray) -> np.ndarray:
    out, _ = run(x)
    return out


# revision 4
# speedup vs baseline: 1.2012x; 1.0492x over previous
"""LIF spiking-neuron recurrence on Trainium2 (8 NeuronCores).

Reference semantics (TAU=1, THRESH=1, f32):
    mem = 0
    for t in range(T):
        mem = mem + x[t]
        spike[t] = (mem >= 1.0) ? 1.0 : 0.0
        mem = mem * (1 - spike[t])        # hard reset

Sharding: data-parallel over batch (B=128 -> 16 rows/core); per-core
[T, 16, 16384] viewed as [T, 128, 2048], host-transposed to [128, T, 2048].

Two tricks make this DMA-input-bound (~70MB/core at ~350GB/s):

1. Fused recurrence op. A runtime-registered custom DVE op folds the
   previous step's reset into this step's add, so the recurrence is ONE
   1x DVE pass/step (2.29us) instead of TT+STT (3.5us):

       tmp_t = select(tmp_{t-1} < 1, tmp_{t-1}, 0) + x_t   (LIF_STEP_ANT)

   tmp_t is the pre-reset membrane; mem_t is never materialized.
   Bit-exact: select passes tmp through untouched (incl -0.0) below
   threshold and yields +0.0 at/above it, matching tmp*(1-spike) in f32.
   ACT computes mask_t = [tmp_t < 1] as u8 via Sign(1 - tmp_t) (negative
   Sign saturates to u8 0; Sign(0)=0 handles the exact tie tmp==1).

2. Bit-packed output (16MiB -> 2MiB/core). The idle PE packs 8 steps of
   masks into one byte: mask u8 {0,1} BITCAST to fp8e4 reads as
   {0, 2^-9} (denormal; probed NOT flushed by the PE), matmul'd against
   stationary diag(2^(9+k)) bf16 weights with PSUM f32 accumulation over
   k=0..7 -> exact integers 0..255. DVE tensor_copy extracts PSUM f32 ->
   SBUF u8 once per 8 steps (probed exact). Host unpacks bits.

Budget/step: DVE 2.29 + 0.3(extract/8) us, ACT 2.0us, PE ~0.9us, all
under the ~3.1us DMA-in pace. 84MB -> 69MB total DMA per core.
"""

import numpy as np

try:
    import concourse  # noqa: F401
except ImportError:  # pragma: no cover
    import sys

    for _p in ("/opt/trn_rl_repo", "/root/.axon_site/_ro/trn_rl_repo"):
        if _p not in sys.path:
            sys.path.insert(0, _p)

import ml_dtypes

from concourse import bacc, mybir
from concourse.bass_utils import run_bass_kernel_spmd
from concourse.mybir import ActivationFunctionType as AF
from concourse.tile import TileContext

T, B, D = 64, 128, 16384
NCORES = 8
BL = B // NCORES  # 16 batch rows per core
P = 128
F = (BL * D) // P  # 2048
PK = 8  # timesteps packed per output byte
NG = T // PK  # output groups


def register_lif_op():
    """Runtime-register the fused LIF-step DVE op:
    out = select(in0 < s0, in0, 0) + in1  (s0 = threshold)."""
    from concourse import dve_ops
    from concourse.dve_ops import (
        OPS,
        DveOp,
        _CUSTOM_DVE_ROW_BASE,
        _SUB_OPCODE_FOR_NAME,
    )
    from concourse.dve_spec import C0, Spec, Src0, Src1, Zero, select

    if "LIF_STEP_ANT" in _SUB_OPCODE_FOR_NAME:
        return next(op for op in OPS if op.name == "LIF_STEP_ANT")

    spec = Spec(
        body=select(Src0 < C0, Src0, Zero) + Src1,
        reference=lambda in0, in1, s0, s1, imm2: (
            np.where(in0 < s0, in0, np.float32(0.0)) + in1
        ).astype(np.float32),
    )
    op = DveOp(
        "LIF_STEP_ANT",
        spec,
        subdim=False,
        uops_sha={"v3": "38f6b55dbeb193f6", "v4": "cb4fb9e0c41a0972"},
    )
    OPS.append(op)
    _SUB_OPCODE_FOR_NAME[op.name] = _CUSTOM_DVE_ROW_BASE + len(OPS) - 1
    dve_ops.CUSTOM_DVE_SPECS[op.name] = op.spec
    return op


def build_nc(t_steps=T, x_chunk=4, x_bufs=3):
    assert t_steps % PK == 0
    f32 = mybir.dt.float32
    u8 = mybir.dt.uint8
    bf16 = mybir.dt.bfloat16
    fp8e4 = mybir.dt.float8e4
    lif = register_lif_op()
    nc = bacc.Bacc(
        "TRN2", target_bir_lowering=False, debug=False, num_devices=NCORES
    )
    x_ext = nc.dram_tensor("x", [P, t_steps, F], f32, kind="ExternalInput")
    w_ext = nc.dram_tensor("w", [PK, P, P], bf16, kind="ExternalInput")
    n_groups_out = t_steps // PK
    out_ext = nc.dram_tensor(
        "out", [P, n_groups_out, F], u8, kind="ExternalOutput"
    )
    n_xgroups = (t_steps + x_chunk - 1) // x_chunk
    with TileContext(nc) as tc:
        with (
            tc.tile_pool(name="mp", bufs=1) as mp,
            tc.tile_pool(name="tp", bufs=3) as tp,
            tc.tile_pool(name="kp", bufs=3) as kp,
            tc.tile_pool(name="xp", bufs=x_bufs) as xp,
            tc.tile_pool(name="op", bufs=2) as op_pool,
            tc.psum_pool(name="pp", bufs=2) as pp,
        ):
            b1 = mp.tile([P, 1], f32, name="b1")
            nc.vector.memset(b1[:], 1.0)
            zero = mp.tile([P, F], f32, name="zero")
            nc.vector.memset(zero[:], 0.0)
            ws = [mp.tile([P, P], bf16, name=f"w{k}") for k in range(PK)]
            for k in range(PK):
                nc.sync.dma_start(ws[k][:], w_ext[k, :, :])

            x_tiles = {}

            def x_slab(t):
                g, j = divmod(t, x_chunk)
                return x_tiles[g][:, j * F : (j + 1) * F]

            def ensure_x_loaded(g):
                if g in x_tiles or g >= n_xgroups:
                    return
                xt = xp.tile([P, x_chunk * F], f32, name="xt")
                x_tiles[g] = xt
                xv = x_ext[:, g * x_chunk : (g + 1) * x_chunk, :]
                for j in range(x_chunk):
                    nc.sync.dma_start(
                        xt[:, j * F : (j + 1) * F], xv[:, j, :]
                    )

            prev = zero
            for og in range(n_groups_out):
                ps = pp.tile([P, F], f32, name="ps")
                for k in range(PK):
                    t = og * PK + k
                    xg = t // x_chunk
                    ensure_x_loaded(xg)
                    ensure_x_loaded(xg + 1)
                    tmp = tp.tile([P, F], f32, name="tmp")
                    # tmp_t = select(tmp_{t-1} < 1, tmp_{t-1}, 0) + x_t
                    nc.vector._custom_dve(
                        lif, out=tmp[:], in0=prev[:], in1=x_slab(t), s0=1.0
                    )
                    # mask_t = [tmp_t < 1] as u8 (Sign(1-tmp); <0 saturates)
                    mk = kp.tile([P, F], u8, name="mk")
                    nc.scalar.activation(
                        mk[:], tmp[:], AF.Sign, bias=b1[:], scale=-1.0
                    )
                    # pack: ps += 2^(9+k) * fp8e4(mask)  (denorm 2^-9 * 2^(9+k));
                    # one MM per 512-col PSUM bank
                    for c in range(F // 512):
                        sl = slice(c * 512, (c + 1) * 512)
                        nc.tensor.matmul(
                            ps[:, sl],
                            ws[k][:],
                            mk[:, sl].bitcast(fp8e4),
                            start=(k == 0),
                            stop=(k == PK - 1),
                        )
                    prev = tmp
                # extract packed byte: f32 ints 0..255 -> u8
                pk_t = op_pool.tile([P, F], u8, name="pk")
                nc.vector.tensor_copy(pk_t[:], ps[:])
                nc.scalar.dma_start(out_ext[:, og, :], pk_t[:])
    nc.compile()
    return nc


_cached_nc = None


def _get_nc():
    global _cached_nc
    if _cached_nc is None:
        _cached_nc = build_nc()
    return _cached_nc


def _pack_weights():
    w = np.zeros((PK, P, P), dtype=ml_dtypes.bfloat16)
    for k in range(PK):
        np.fill_diagonal(w[k], np.float32(2.0 ** (9 + k)))
    return w


_W = _pack_weights()


def _shard(x):
    in_maps = []
    for c in range(NCORES):
        xc = x[:, c * BL : (c + 1) * BL, :].reshape(T, P, F).transpose(1, 0, 2)
        in_maps.append({"x": np.ascontiguousarray(xc), "w": _W})
    return in_maps


def _gather(results):
    shifts = np.arange(PK, dtype=np.uint8)[None, None, :, None]
    outs = []
    for c in range(NCORES):
        pk = np.asarray(results[c]["out"])  # [P, NG, F] packed mask bits
        bits = (pk[:, :, None, :] >> shifts) & 1  # [P, NG, PK, F] mask
        spikes = (1 - bits).astype(np.float32).reshape(P, T, F)
        outs.append(spikes.transpose(1, 0, 2).reshape(T, BL, D))
    return np.concatenate(outs, axis=1)


def run(x, trace=False, **kw):
    x = np.ascontiguousarray(np.asarray(x, dtype=np.float32))
    assert x.shape == (T, B, D), x.shape
    nc = _get_nc()
    res = run_bass_kernel_spmd(
        nc, _shard(x), core_ids=list(range(NCORES)), trace=trace, **kw
    )
    return _gather(res.results), res


def kernel(x: np.ndarray) -> np.ndarray:
    out, _ = run(x)
    return out


# revision 5
# speedup vs baseline: 1.5295x; 1.2732x over previous
"""LIF spiking-neuron recurrence on Trainium2 (8 NeuronCores).

Reference semantics (TAU=1, THRESH=1, f32):
    mem = 0
    for t in range(T):
        mem = mem + x[t]
        spike[t] = (mem >= 1.0) ? 1.0 : 0.0
        mem = mem * (1 - spike[t])        # hard reset

Sharding: data-parallel over batch (B=128 -> 16 rows/core); per-core
[T, 16, 16384] viewed as [T, 128, 2048], host-transposed to [128, T, 2048].

Two tricks make this DMA-input-bound (~70MB/core at ~350GB/s):

1. Fused recurrence op. A runtime-registered custom DVE op folds the
   previous step's reset into this step's add, so the recurrence is ONE
   1x DVE pass/step (2.29us) instead of TT+STT (3.5us):

       tmp_t = select(tmp_{t-1} < 1, tmp_{t-1}, 0) + x_t   (LIF_STEP_ANT)

   tmp_t is the pre-reset membrane; mem_t is never materialized.
   Bit-exact: select passes tmp through untouched (incl -0.0) below
   threshold and yields +0.0 at/above it, matching tmp*(1-spike) in f32.
   ACT computes mask_t = [tmp_t < 1] as u8 via Sign(1 - tmp_t) (negative
   Sign saturates to u8 0; Sign(0)=0 handles the exact tie tmp==1).

2. Bit-packed output (16MiB -> 2MiB/core). The idle PE packs 8 steps of
   masks into one byte: mask u8 {0,1} BITCAST to fp8e4 reads as
   {0, 2^-9} (denormal; probed NOT flushed by the PE), matmul'd against
   stationary diag(2^(9+k)) bf16 weights with PSUM f32 accumulation over
   k=0..7 -> exact integers 0..255. DVE tensor_copy extracts PSUM f32 ->
   SBUF u8 once per 8 steps (probed exact). Host unpacks bits.

Budget/step: DVE 2.29 + 0.3(extract/8) us, ACT 2.0us, PE ~0.9us, all
under the ~3.1us DMA-in pace. 84MB -> 69MB total DMA per core.
"""

import numpy as np

try:
    import concourse  # noqa: F401
except ImportError:  # pragma: no cover
    import sys

    for _p in ("/opt/trn_rl_repo", "/root/.axon_site/_ro/trn_rl_repo"):
        if _p not in sys.path:
            sys.path.insert(0, _p)

import ml_dtypes

from concourse import bacc, mybir
from concourse.bass_utils import run_bass_kernel_spmd
from concourse.mybir import ActivationFunctionType as AF
from concourse.tile import TileContext

T, B, D = 64, 128, 16384
NCORES = 8
BL = B // NCORES  # 16 batch rows per core
P = 128
F = (BL * D) // P  # 2048
PK = 8  # timesteps packed per output byte
NG = T // PK  # output groups


def register_lif_op():
    """Runtime-register the fused LIF-step DVE op:
    out = select(in0 < s0, in0, 0) + in1  (s0 = threshold)."""
    from concourse import dve_ops
    from concourse.dve_ops import (
        OPS,
        DveOp,
        _CUSTOM_DVE_ROW_BASE,
        _SUB_OPCODE_FOR_NAME,
    )
    from concourse.dve_spec import C0, Spec, Src0, Src1, Zero, select

    if "LIF_STEP_ANT" in _SUB_OPCODE_FOR_NAME:
        return next(op for op in OPS if op.name == "LIF_STEP_ANT")

    spec = Spec(
        body=select(Src0 < C0, Src0, Zero) + Src1,
        reference=lambda in0, in1, s0, s1, imm2: (
            np.where(in0 < s0, in0, np.float32(0.0)) + in1
        ).astype(np.float32),
    )
    op = DveOp(
        "LIF_STEP_ANT",
        spec,
        subdim=False,
        uops_sha={"v3": "38f6b55dbeb193f6", "v4": "cb4fb9e0c41a0972"},
    )
    OPS.append(op)
    _SUB_OPCODE_FOR_NAME[op.name] = _CUSTOM_DVE_ROW_BASE + len(OPS) - 1
    dve_ops.CUSTOM_DVE_SPECS[op.name] = op.spec
    return op


def build_nc(t_steps=T, x_chunk=4, x_bufs=3):
    assert t_steps % PK == 0
    f32 = mybir.dt.float32
    u8 = mybir.dt.uint8
    bf16 = mybir.dt.bfloat16
    fp8e4 = mybir.dt.float8e4
    lif = register_lif_op()
    nc = bacc.Bacc(
        "TRN2", target_bir_lowering=False, debug=False, num_devices=NCORES
    )
    x_ext = nc.dram_tensor("x", [P, t_steps, F], f32, kind="ExternalInput")
    w_ext = nc.dram_tensor("w", [PK, P, P], bf16, kind="ExternalInput")
    n_groups_out = t_steps // PK
    out_ext = nc.dram_tensor(
        "out", [P, n_groups_out, F], u8, kind="ExternalOutput"
    )
    n_xgroups = (t_steps + x_chunk - 1) // x_chunk
    with TileContext(nc) as tc:
        with (
            tc.tile_pool(name="mp", bufs=1) as mp,
            tc.tile_pool(name="tp", bufs=3) as tp,
            tc.tile_pool(name="kp", bufs=3) as kp,
            tc.tile_pool(name="xp", bufs=x_bufs) as xp,
            tc.tile_pool(name="op", bufs=2) as op_pool,
            tc.psum_pool(name="pp", bufs=2) as pp,
        ):
            b1 = mp.tile([P, 1], f32, name="b1")
            nc.vector.memset(b1[:], 1.0)
            zero = mp.tile([P, F], f32, name="zero")
            nc.vector.memset(zero[:], 0.0)
            ws = [mp.tile([P, P], bf16, name=f"w{k}") for k in range(PK)]
            for k in range(PK):
                nc.sync.dma_start(ws[k][:], w_ext[k, :, :])

            x_tiles = {}

            def x_slab(t):
                g, j = divmod(t, x_chunk)
                return x_tiles[g][:, j * F : (j + 1) * F]

            def ensure_x_loaded(g):
                if g in x_tiles or g >= n_xgroups:
                    return
                xt = xp.tile([P, x_chunk * F], f32, name="xt")
                x_tiles[g] = xt
                xv = x_ext[:, g * x_chunk : (g + 1) * x_chunk, :]
                for j in range(x_chunk):
                    nc.sync.dma_start(
                        xt[:, j * F : (j + 1) * F], xv[:, j, :]
                    )

            prev = zero
            for og in range(n_groups_out):
                ps = pp.tile([P, F], f32, name="ps")
                for k in range(PK):
                    t = og * PK + k
                    xg = t // x_chunk
                    ensure_x_loaded(xg)
                    ensure_x_loaded(xg + 1)
                    tmp = tp.tile([P, F], f32, name="tmp")
                    # tmp_t = select(tmp_{t-1} < 1, tmp_{t-1}, 0) + x_t
                    nc.vector._custom_dve(
                        lif, out=tmp[:], in0=prev[:], in1=x_slab(t), s0=1.0
                    )
                    # mask_t = [tmp_t < 1] as u8 (Sign(1-tmp); <0 saturates)
                    mk = kp.tile([P, F], u8, name="mk")
                    nc.scalar.activation(
                        mk[:], tmp[:], AF.Sign, bias=b1[:], scale=-1.0
                    )
                    # pack: ps += 2^(9+k) * fp8e4(mask)  (denorm 2^-9 * 2^(9+k));
                    # one MM per 512-col PSUM bank
                    for c in range(F // 512):
                        sl = slice(c * 512, (c + 1) * 512)
                        nc.tensor.matmul(
                            ps[:, sl],
                            ws[k][:],
                            mk[:, sl].bitcast(fp8e4),
                            start=(k == 0),
                            stop=(k == PK - 1),
                        )
                    prev = tmp
                # extract packed byte: f32 ints 0..255 -> u8 (on ACT; DVE is
                # saturated by the LIF ops, ACT has slack — probed exact)
                pk_t = op_pool.tile([P, F], u8, name="pk")
                nc.scalar.activation(pk_t[:], ps[:], AF.Copy)
                nc.scalar.dma_start(out_ext[:, og, :], pk_t[:])
    nc.compile()
    return nc


_cached_nc = None


def _get_nc():
    global _cached_nc
    if _cached_nc is None:
        _cached_nc = build_nc()
    return _cached_nc


def _pack_weights():
    w = np.zeros((PK, P, P), dtype=ml_dtypes.bfloat16)
    for k in range(PK):
        np.fill_diagonal(w[k], np.float32(2.0 ** (9 + k)))
    return w


_W = _pack_weights()


def _shard(x):
    in_maps = []
    for c in range(NCORES):
        xc = x[:, c * BL : (c + 1) * BL, :].reshape(T, P, F).transpose(1, 0, 2)
        in_maps.append({"x": np.ascontiguousarray(xc), "w": _W})
    return in_maps


def _gather(results):
    shifts = np.arange(PK, dtype=np.uint8)[None, None, :, None]
    outs = []
    for c in range(NCORES):
        pk = np.asarray(results[c]["out"])  # [P, NG, F] packed mask bits
        bits = (pk[:, :, None, :] >> shifts) & 1  # [P, NG, PK, F] mask
        spikes = (1 - bits).astype(np.float32).reshape(P, T, F)
        outs.append(spikes.transpose(1, 0, 2).reshape(T, BL, D))
    return np.concatenate(outs, axis=1)


def run(x, trace=False, **kw):
    x = np.ascontiguousarray(np.asarray(x, dtype=np.float32))
    assert x.shape == (T, B, D), x.shape
    nc = _get_nc()
    res = run_bass_kernel_spmd(
        nc, _shard(x), core_ids=list(range(NCORES)), trace=trace, **kw
    )
    return _gather(res.results), res


def kernel(x: np.ndarray) -> np.ndarray:
    out, _ = run(x)
    return out
